# revision 43
# baseline (speedup 1.0000x reference)
"""FPCA window attention kernel for 8 Trainium2 NeuronCores.

Strategy: data-parallel over batch (B=16 -> 2 per core); windows are fully
independent. Per core we process 2 batches x 16 window-rows x 16 windows.

Layouts (host-prepped):
  - q/k/v pre-transposed to feature-major [B, nH, C, nW*L] so projection
    matmuls need no on-device transpose.
  - logit scale exp(min(ls, ln100)) and all biases folded into weights:
      WqT_s = Wq.T * scale (per-head out-col scale), bq_s = bq * scale
      bv folded via softmax-rows-sum-1:  bp_eff = bp + Wp @ bv
  - position pre-arranged to the on-device S-tile layout.
  - outputs written in device-friendly layouts, host transposes back.

Device pipeline per window-row (16 windows, processed as 4 "2-pair" tiles):
  proj (bf16 matmuls, weights stationary) -> S = qp.kp^T per head via
  PE-subtile-packed K=32 matmuls, position pre-loaded into PSUM via an
  identity matmul (float32r) -> exp (no max-subtraction; logits bounded)
  -> row-sums + reciprocal + normalize -> attn DMA out (fp32)
  -> PE transpose of attn -> AV matmuls (packed 64x32 subtiles)
  -> output projection -> x DMA out (feature-major fp32).
"""

import sys
import types

import numpy as np

sys.path.insert(0, "/opt/trn_rl_repo")

import ml_dtypes  # noqa: E402

# --- register the NTFF profile hook that this container's antenv lacks ---
try:  # pragma: no cover - only matters when tracing
    import antenv.axon_hooks  # noqa: F401
except Exception:
    try:
        from trn_agent_boot.trn_boot import _ntff_profile_via_ctypes

        _hook = _ntff_profile_via_ctypes("/opt/axon/libaxon_pjrt.so")
        _mod = types.ModuleType("antenv.axon_hooks")
        _mod.get_axon_ntff_profile_hook = lambda: _hook
        _mod.set_axon_ntff_profile_hook = lambda h: None
        sys.modules["antenv.axon_hooks"] = _mod
    except Exception:
        pass

import concourse.bacc as bacc  # noqa: E402
import concourse.bass as bass  # noqa: E402
import concourse.tile as tile  # noqa: E402
from concourse import bass_utils as _bu  # noqa: E402
from concourse import mybir  # noqa: E402
from concourse.bass_utils import run_bass_kernel_spmd  # noqa: E402

# note: --enable-ldw-opt=true breaks walrus codegen (visitInstLdweights
# error), so the serialized-LDWEIGHTS default stays.

B, nH, nW, L, C, H = 16, 16, 16, 64, 128, 4
d = C // H  # 32
NCORES = 8
B_loc = B // NCORES  # 2
LOGIT_MAX = float(np.log(1.0 / 0.01))

F32 = mybir.dt.float32
F32R = mybir.dt.float32r
BF16 = mybir.dt.bfloat16
BF = ml_dtypes.bfloat16

_compiled = {}

POS_VIA_MATMUL = True  # False: DVE tensor_add for position (slower, safer)


def _build(rows=None, pos_via_matmul=None):
    """Build + compile the per-core Bass program. rows: list of (b, i)."""
    if pos_via_matmul is None:
        pos_via_matmul = POS_VIA_MATMUL
    if rows is None:
        rows = [(b, i) for b in range(B_loc) for i in range(nH)]

    nc = bacc.Bacc()

    # ---- dram tensors ----
    qT = nc.dram_tensor("qT", [B_loc, nH, C, nW * L], F32, kind="ExternalInput")
    kT = nc.dram_tensor("kT", [B_loc, nH, C, nW * L], F32, kind="ExternalInput")
    vT = nc.dram_tensor("vT", [B_loc, nH, C, nW * L], F32, kind="ExternalInput")
    pos = nc.dram_tensor("pos", [B_loc, 128, 512], F32, kind="ExternalInput")
    w_q = nc.dram_tensor("w_q", [C, C], BF16, kind="ExternalInput")
    # k-projection weights with odd/even head output columns zeroed, so
    # K=64 two-head-stacked S matmuls contract only the wanted head
    w_k = nc.dram_tensor("w_k", [C, 2, C], BF16, kind="ExternalInput")
    w_v = nc.dram_tensor("w_v", [C, C], BF16, kind="ExternalInput")
    w_p = nc.dram_tensor("w_p", [C, C], BF16, kind="ExternalInput")
    bq_d = nc.dram_tensor("bq_s", [C, 1], F32, kind="ExternalInput")
    bp_d = nc.dram_tensor("bp_e", [C, 1], F32, kind="ExternalInput")
    eye_d = nc.dram_tensor("eye", [128, 128], F32, kind="ExternalInput")

    # attn stored tile-contiguous: [b, i, t, (h2 lq), (p2 hh w lv)]
    attn_o = nc.dram_tensor(
        "attn_o", [B_loc, nH, 4, 128, 512], F32, kind="ExternalOutput"
    )
    xT_o = nc.dram_tensor("xT_o", [B_loc, nH, C, nW * L], F32, kind="ExternalOutput")

    from contextlib import ExitStack

    with tile.TileContext(nc) as tc, ExitStack() as ctx:
        consts = ctx.enter_context(tc.tile_pool(name="consts", bufs=1))
        qkv_p = ctx.enter_context(tc.tile_pool(name="qkv", bufs=3))
        projo = ctx.enter_context(tc.tile_pool(name="projo", bufs=2))
        attn_p = ctx.enter_context(tc.tile_pool(name="attn", bufs=4))
        small = ctx.enter_context(tc.tile_pool(name="small", bufs=8))
        atT_p = ctx.enter_context(tc.tile_pool(name="atT", bufs=3))
        xsb_p = ctx.enter_context(tc.tile_pool(name="xsb", bufs=4))
        xout_p = ctx.enter_context(tc.tile_pool(name="xout", bufs=3))
        ps_proj = ctx.enter_context(tc.tile_pool(name="ps_proj", bufs=2, space="PSUM"))
        # S tiles die at exp; attnT tiles born at transpose — share slots
        ps_sat = ctx.enter_context(tc.tile_pool(name="ps_sat", bufs=4, space="PSUM"))
        ps_s = ps_sat
        ps_at = ps_sat
        ps_x = ctx.enter_context(tc.tile_pool(name="ps_x", bufs=2, space="PSUM"))

        # ---- constants in SBUF ----
        w_qs = consts.tile([C, C], BF16, tag="wq")
        w_ks = consts.tile([C, 2, C], BF16, tag="wk")
        w_vs = consts.tile([C, C], BF16, tag="wv")
        w_ps_ = consts.tile([C, C], BF16, tag="wp")
        bq_s = consts.tile([C, 1], F32, tag="bq")
        bp_s = consts.tile([C, 1], F32, tag="bp")
        eye_s = consts.tile([128, 128], F32, tag="eye")
        eye_b = consts.tile([128, 128], BF16, tag="eye_b")
        pos_dt = F32R if pos_via_matmul else F32
        if pos_via_matmul:
            eye_r = consts.tile([128, 128], F32R, tag="eye_r")
        pos_s = [
            consts.tile([128, 512], pos_dt, tag=f"pos{b}", name=f"pos_sb{b}")
            for b in range(B_loc)
        ]
        nc.sync.dma_start(out=w_qs, in_=w_q[:, :])
        nc.sync.dma_start(out=w_ks, in_=w_k[:, :, :])
        nc.sync.dma_start(out=w_vs, in_=w_v[:, :])
        nc.sync.dma_start(out=w_ps_, in_=w_p[:, :])
        nc.sync.dma_start(out=bq_s, in_=bq_d[:, :])
        nc.sync.dma_start(out=bp_s, in_=bp_d[:, :])
        nc.sync.dma_start(out=eye_s, in_=eye_d[:, :])
        nc.gpsimd.dma_start(out=eye_b, in_=eye_d[:, :])
        if pos_via_matmul:
            nc.gpsimd.dma_start(out=eye_r, in_=eye_d[:, :])
        for b in range(B_loc):
            nc.gpsimd.dma_start(out=pos_s[b], in_=pos[b])

        for b, i in rows:
            # ---- load row (16 windows), casting fp32 -> bf16 in-DMA ----
            q_sb = qkv_p.tile([128, nW * L], BF16, tag="q")
            k_sb = qkv_p.tile([128, nW * L], BF16, tag="k")
            v_sb = qkv_p.tile([128, nW * L], BF16, tag="v")
            nc.gpsimd.dma_start(out=q_sb, in_=qT[b, i])
            nc.gpsimd.dma_start(out=k_sb, in_=kT[b, i])
            nc.gpsimd.dma_start(out=v_sb, in_=vT[b, i])

            # ---- projections ----
            qpT = projo.tile([128, nW * L], BF16, tag="qpT")
            kpT_m = [
                projo.tile([128, nW * L], BF16, tag=f"kpT{m}", name=f"kpT_m{m}")
                for m in range(2)
            ]
            vp = projo.tile([128, nW * L], BF16, tag="vp")
            for hf in range(2):
                sl = slice(hf * 512, hf * 512 + 512)
                qp_ps = ps_proj.tile([128, 512], F32, tag="pp")
                nc.tensor.matmul(qp_ps, w_qs, q_sb[:, sl], start=True, stop=True)
                nc.scalar.activation(
                    qpT[:, sl], qp_ps, mybir.ActivationFunctionType.Identity,
                    bias=bq_s,
                )
                for m in range(2):
                    kp_ps = ps_proj.tile([128, 512], F32, tag="pp")
                    nc.tensor.matmul(
                        kp_ps, w_ks[:, m, :], k_sb[:, sl], start=True, stop=True
                    )
                    nc.scalar.activation(
                        kpT_m[m][:, sl], kp_ps, mybir.ActivationFunctionType.Copy
                    )
            for g in range(2):  # vp: natural layout, per pair of windows
                vp_ps = ps_proj.tile([128, 512], F32, tag="pp")
                for p in range(4):
                    pair = g * 4 + p
                    nc.tensor.matmul(
                        vp_ps[:, p * 128 : p * 128 + 128],
                        v_sb[:, pair * 128 : pair * 128 + 128],
                        w_vs,
                        start=True,
                        stop=True,
                    )
                nc.vector.tensor_copy(vp[:, g * 512 : g * 512 + 512], vp_ps)

            attnT_t = {}
            for t in range(4):  # 2-pair S tiles (windows 4t..4t+3)
                s_ps = ps_s.tile([128, 512], F32, tag="sat")
                if pos_via_matmul:
                    # position pre-load (f32r identity matmul, full bank)
                    nc.tensor.matmul(
                        s_ps,
                        eye_r,
                        pos_s[b],
                        start=True,
                        stop=False,
                    )
                # S layout: p = 64*hB + lq, f = p2*256 + 128*hL + 64*w + lv,
                # head h = 2*hB + hL. All matmuls on diagonal 64x64 tiles:
                # K=64 (two heads stacked), masked k-weights kill the
                # sibling head's contribution.
                for hB in range(2):
                    rp = slice(64 * hB, 64 * hB + 64)
                    for hL in range(2):
                        for p2 in range(2):
                            pair = 2 * t + p2
                            for w in range(2):
                                is_start = (
                                    not pos_via_matmul
                                    and hL == 0
                                    and p2 == 0
                                    and w == 0
                                )
                                is_stop = hL == 1 and p2 == 1 and w == 1
                                csl = slice(
                                    pair * 128 + w * 64, pair * 128 + w * 64 + 64
                                )
                                out = s_ps[
                                    rp,
                                    p2 * 256 + 128 * hL + 64 * w : p2 * 256
                                    + 128 * hL
                                    + 64 * w
                                    + 64,
                                ]
                                nc.tensor.matmul(
                                    out,
                                    qpT[rp, csl],
                                    kpT_m[hL][rp, csl],
                                    start=is_start,
                                    stop=is_stop,
                                    skip_group_check=True,
                                    tile_position=(64 * hB, 64 * hB),
                                )
                # softmax (no max subtraction: |logits| <~ 25, exp safe in fp32)
                at = attn_p.tile([128, 512], F32, tag="at")
                if pos_via_matmul:
                    nc.scalar.activation(at, s_ps, mybir.ActivationFunctionType.Exp)
                else:
                    nc.vector.tensor_add(at, s_ps, pos_s[b])
                    nc.scalar.activation(at, at, mybir.ActivationFunctionType.Exp)
                sums = small.tile([128, 8], F32, tag="sums")
                nc.vector.reduce_sum(
                    out=sums,
                    in_=at.rearrange("p (g l) -> p g l", l=64),
                    axis=mybir.AxisListType.X,
                )
                recip = small.tile([128, 8], F32, tag="recip")
                nc.vector.reciprocal(recip, sums)
                # one multiply with the reciprocal broadcast along lv
                rb = bass.AP(
                    tensor=recip.tensor,
                    offset=recip.offset,
                    ap=[recip.ap[0], recip.ap[1], [0, 64]],
                )
                at3 = at.rearrange("p (g l) -> p g l", l=64)
                nc.vector.tensor_mul(at3, at3, rb)
                # attn out DMA: tile-contiguous, host unscrambles
                nc.sync.dma_start(out=attn_o[b, i, t], in_=at)
                # transpose attn for AV
                at_ps = ps_at.tile([128, 512], F32, tag="sat", name="at_ps")
                for cix in range(4):
                    sl = slice(cix * 128, cix * 128 + 128)
                    nc.tensor.transpose(at_ps[:, sl], at[:, sl], eye_s)
                aT = atT_p.tile([128, 512], BF16, tag="aT")
                nc.scalar.activation(aT, at_ps, mybir.ActivationFunctionType.Copy)
                attnT_t[t] = aT

            for g in range(2):  # AV + out-proj per 4 pairs (8 windows)
                # AV with x in natural layout: diagonal 64x64 tiles
                # (32-wide column tiling is broken on this toolchain/hw)
                x_ps = ps_x.tile([128, 512], F32, tag="x")
                for p4 in range(4):
                    pair = 4 * g + p4
                    t, p2 = pair // 2, pair % 2
                    aT = attnT_t[t]
                    for w in range(2):
                        for h in range(4):
                            lhsT = aT[
                                64 * w : 64 * w + 64,
                                128 * (2 * p2 + h % 2)
                                + 64 * (h // 2) : 128 * (2 * p2 + h % 2)
                                + 64 * (h // 2)
                                + 64,
                            ]
                            rhs = vp[
                                64 * w : 64 * w + 64,
                                pair * 128 + 32 * h : pair * 128 + 32 * h + 32,
                            ]
                            out = x_ps[
                                64 * w : 64 * w + 64,
                                p4 * 128 + 32 * h : p4 * 128 + 32 * h + 32,
                            ]
                            nc.tensor.matmul(
                                out,
                                lhsT,
                                rhs,
                                start=(p4 == 0 and h == 0),
                                stop=(p4 == 3 and h == 3),
                                skip_group_check=True,
                                tile_position=(64 * w, 64 * w),
                            )
                x_sb = xsb_p.tile([128, 512], BF16, tag="x_sb")
                nc.vector.tensor_copy(x_sb, x_ps)
                # transpose x -> feature-major xT for the output projection
                xt_ps = ps_x.tile([128, 512], BF16, tag="x")
                for p4 in range(4):
                    sl = slice(p4 * 128, p4 * 128 + 128)
                    nc.tensor.transpose(xt_ps[:, sl], x_sb[:, sl], eye_b)
                xT_sb = xsb_p.tile([128, 512], BF16, tag="xT")
                nc.scalar.activation(
                    xT_sb, xt_ps, mybir.ActivationFunctionType.Copy
                )
                xo_ps = ps_x.tile([128, 512], F32, tag="x")
                nc.tensor.matmul(xo_ps, w_ps_, xT_sb, start=True, stop=True)
                xo_sb = xout_p.tile([128, 512], F32, tag="xo")
                nc.scalar.activation(
                    xo_sb, xo_ps, mybir.ActivationFunctionType.Identity, bias=bp_s
                )
                nc.sync.dma_start(
                    out=xT_o[b, i, :, g * 512 : g * 512 + 512], in_=xo_sb
                )

    nc.compile()
    return nc


def _get_nc():
    if "nc" not in _compiled:
        _compiled["nc"] = _build()
    return _compiled["nc"]


def _prep_inputs(query, key, value, position, Wq, bq, Wk, Wv, bv, Wp, bp, logit_scale):
    """Host-side prep -> list of per-core input maps."""
    scale = np.exp(np.minimum(np.asarray(logit_scale, np.float32), LOGIT_MAX)).reshape(
        H
    )  # [H]
    col_scale = np.repeat(scale, d)  # per C_out column
    WqT_s = (np.asarray(Wq, np.float32).T * col_scale[None, :]).astype(BF)
    bq_s = (np.asarray(bq, np.float32) * col_scale).astype(np.float32).reshape(C, 1)
    WkT_f = np.asarray(Wk, np.float32).T
    head_par = (np.arange(C) // d) % 2  # parity of each C_out column's head
    WkT = np.stack(
        [WkT_f * (head_par == m)[None, :] for m in range(2)], axis=1
    ).astype(BF)  # [C_in, 2, C_out]
    WvT = np.asarray(Wv, np.float32).T.astype(BF)
    WpT = np.asarray(Wp, np.float32).T.astype(BF)
    bp_eff = (
        (np.asarray(bp, np.float32) + np.asarray(Wp, np.float32) @ np.asarray(bv, np.float32))
        .astype(np.float32)
        .reshape(C, 1)
    )
    eye = np.eye(128, dtype=np.float32)

    def featmaj(x):  # [B,nH,nW,L,C] -> [B,nH,C,nW*L]
        return (
            np.ascontiguousarray(np.asarray(x, np.float32).transpose(0, 1, 4, 2, 3))
            .reshape(B, nH, C, nW * L)
        )

    qT = featmaj(query)
    kT = featmaj(key)
    vT = featmaj(value)

    # position -> [B, (hB lq)=128, (p2 hL w lv)=512], head h = 2*hB + hL
    p4 = np.asarray(position, np.float32).reshape(B, 2, 2, L, L)  # b, hB, hL, lq, lv
    core = p4.transpose(0, 1, 3, 2, 4).reshape(B, 128, 2, 1, L)  # b,(hB lq),hL,w,lv
    core = np.broadcast_to(core, (B, 128, 2, 2, L)).reshape(B, 128, 256)
    pos_arr = np.ascontiguousarray(np.tile(core, (1, 1, 2)))  # [B,128,512]

    in_maps = []
    for c in range(NCORES):
        bs = slice(c * B_loc, (c + 1) * B_loc)
        in_maps.append(
            {
                "qT": np.ascontiguousarray(qT[bs]),
                "kT": np.ascontiguousarray(kT[bs]),
                "vT": np.ascontiguousarray(vT[bs]),
                "pos": np.ascontiguousarray(pos_arr[bs]),
                "w_q": WqT_s,
                "w_k": WkT,
                "w_v": WvT,
                "w_p": WpT,
                "bq_s": bq_s,
                "bp_e": bp_eff,
                "eye": eye,
            }
        )
    return in_maps


def _assemble(results):
    """Per-core outputs -> full (x, attn)."""
    attn_tmp = np.concatenate([r["attn_o"] for r in results], axis=0)
    xT = np.concatenate([r["xT_o"] for r in results], axis=0)
    # attn_tmp [B,nH,t,(hB lq),(p2 hL w lv)] -> [B,nH,(t p2 w),(hB hL),lq,lv]
    a9 = attn_tmp.reshape(B, nH, 4, 2, L, 2, 2, 2, L)  # b,i,t,hB,lq,p2,hL,w,lv
    attn = np.ascontiguousarray(
        a9.transpose(0, 1, 2, 5, 7, 3, 6, 4, 8)  # b,i,t,p2,w,hB,hL,lq,lv
    ).reshape(B, nH, nW, H, L, L)
    x = np.ascontiguousarray(
        xT.reshape(B, nH, C, nW, L).transpose(0, 1, 3, 4, 2)
    )
    return x, attn


def run(in_maps, trace=False, tmpdir=None):
    nc = _get_nc()
    return run_bass_kernel_spmd(
        nc, in_maps, core_ids=list(range(NCORES)), trace=trace, tmpdir=tmpdir
    )


def kernel(query, key, value, position, Wq, bq, Wk, Wv, bv, Wp, bp, logit_scale):
    in_maps = _prep_inputs(
        query, key, value, position, Wq, bq, Wk, Wv, bv, Wp, bp, logit_scale
    )
    res = run(in_maps)
    return _assemble(res.results)


# revision 46
# speedup vs baseline: 1.6767x; 1.6767x over previous
"""FPCA window attention kernel for 8 Trainium2 NeuronCores.

Strategy: data-parallel over batch (B=16 -> 2 per core); windows are fully
independent. Per core we process 2 batches x 16 window-rows x 16 windows.

Layouts (host-prepped):
  - q/k/v pre-transposed to feature-major [B, nH, C, nW*L] so projection
    matmuls need no on-device transpose.
  - logit scale exp(min(ls, ln100)) and all biases folded into weights:
      WqT_s = Wq.T * scale (per-head out-col scale), bq_s = bq * scale
      bv folded via softmax-rows-sum-1:  bp_eff = bp + Wp @ bv
  - position pre-arranged to the on-device S-tile layout.
  - outputs written in device-friendly layouts, host transposes back.

Device pipeline per window-row (16 windows, processed as 4 "2-pair" tiles):
  proj (bf16 matmuls, weights stationary) -> S = qp.kp^T per head via
  PE-subtile-packed K=32 matmuls, position pre-loaded into PSUM via an
  identity matmul (float32r) -> exp (no max-subtraction; logits bounded)
  -> row-sums + reciprocal + normalize -> attn DMA out (fp32)
  -> PE transpose of attn -> AV matmuls (packed 64x32 subtiles)
  -> output projection -> x DMA out (feature-major fp32).
"""

import sys
import types

import numpy as np

sys.path.insert(0, "/opt/trn_rl_repo")

import ml_dtypes  # noqa: E402

# --- register the NTFF profile hook that this container's antenv lacks ---
try:  # pragma: no cover - only matters when tracing
    import antenv.axon_hooks  # noqa: F401
except Exception:
    try:
        from trn_agent_boot.trn_boot import _ntff_profile_via_ctypes

        _hook = _ntff_profile_via_ctypes("/opt/axon/libaxon_pjrt.so")
        _mod = types.ModuleType("antenv.axon_hooks")
        _mod.get_axon_ntff_profile_hook = lambda: _hook
        _mod.set_axon_ntff_profile_hook = lambda h: None
        sys.modules["antenv.axon_hooks"] = _mod
    except Exception:
        pass

import concourse.bacc as bacc  # noqa: E402
import concourse.bass as bass  # noqa: E402
import concourse.tile as tile  # noqa: E402
from concourse import bass_utils as _bu  # noqa: E402
from concourse import mybir  # noqa: E402
from concourse.bass_utils import run_bass_kernel_spmd  # noqa: E402

# note: --enable-ldw-opt=true breaks walrus codegen (visitInstLdweights
# error), so the serialized-LDWEIGHTS default stays.

B, nH, nW, L, C, H = 16, 16, 16, 64, 128, 4
d = C // H  # 32
NCORES = 8
B_loc = B // NCORES  # 2
LOGIT_MAX = float(np.log(1.0 / 0.01))

F32 = mybir.dt.float32
F32R = mybir.dt.float32r
BF16 = mybir.dt.bfloat16
BF = ml_dtypes.bfloat16

_compiled = {}

POS_VIA_MATMUL = True  # False: DVE tensor_add for position (slower, safer)


def _build(rows=None, pos_via_matmul=None):
    """Build + compile the per-core Bass program. rows: list of (b, i)."""
    if pos_via_matmul is None:
        pos_via_matmul = POS_VIA_MATMUL
    if rows is None:
        rows = [(b, i) for b in range(B_loc) for i in range(nH)]

    nc = bacc.Bacc()

    # ---- dram tensors ----
    qT = nc.dram_tensor("qT", [B_loc, nH, C, nW * L], F32, kind="ExternalInput")
    kT = nc.dram_tensor("kT", [B_loc, nH, C, nW * L], F32, kind="ExternalInput")
    vT = nc.dram_tensor("vT", [B_loc, nH, C, nW * L], F32, kind="ExternalInput")
    pos = nc.dram_tensor("pos", [B_loc, 128, 512], F32, kind="ExternalInput")
    w_q = nc.dram_tensor("w_q", [C, C], BF16, kind="ExternalInput")
    # k-projection weights with odd/even head output columns zeroed, so
    # K=64 two-head-stacked S matmuls contract only the wanted head
    w_k = nc.dram_tensor("w_k", [C, 2, C], BF16, kind="ExternalInput")
    w_v = nc.dram_tensor("w_v", [C, C], BF16, kind="ExternalInput")
    w_p = nc.dram_tensor("w_p", [C, C], BF16, kind="ExternalInput")
    bq_d = nc.dram_tensor("bq_s", [C, 1], F32, kind="ExternalInput")
    bp_d = nc.dram_tensor("bp_e", [C, 1], F32, kind="ExternalInput")
    eye_d = nc.dram_tensor("eye", [128, 128], F32, kind="ExternalInput")

    # attn stored tile-contiguous: [b, i, t, (h2 lq), (p2 hh w lv)]
    attn_o = nc.dram_tensor(
        "attn_o", [B_loc, nH, 4, 128, 512], F32, kind="ExternalOutput"
    )
    xT_o = nc.dram_tensor("xT_o", [B_loc, nH, C, nW * L], F32, kind="ExternalOutput")

    from contextlib import ExitStack

    with tile.TileContext(nc) as tc, ExitStack() as ctx:
        consts = ctx.enter_context(tc.tile_pool(name="consts", bufs=1))
        qkv_p = ctx.enter_context(tc.tile_pool(name="qkv", bufs=2))
        projo = ctx.enter_context(tc.tile_pool(name="projo", bufs=2))
        attn_p = ctx.enter_context(tc.tile_pool(name="attn", bufs=3))
        small = ctx.enter_context(tc.tile_pool(name="small", bufs=8))
        atT_p = ctx.enter_context(tc.tile_pool(name="atT", bufs=2))
        xsb_p = ctx.enter_context(tc.tile_pool(name="xsb", bufs=2))
        xout_p = ctx.enter_context(tc.tile_pool(name="xout", bufs=2))
        ps_proj = ctx.enter_context(tc.tile_pool(name="ps_proj", bufs=2, space="PSUM"))
        ps_s = ctx.enter_context(tc.tile_pool(name="ps_s", bufs=2, space="PSUM"))
        ps_at = ctx.enter_context(tc.tile_pool(name="ps_at", bufs=2, space="PSUM"))
        ps_x = ctx.enter_context(tc.tile_pool(name="ps_x", bufs=2, space="PSUM"))

        # ---- constants in SBUF ----
        w_qs = consts.tile([C, C], BF16, tag="wq")
        w_ks = consts.tile([C, 2, C], BF16, tag="wk")
        w_vs = consts.tile([C, C], BF16, tag="wv")
        w_ps_ = consts.tile([C, C], BF16, tag="wp")
        bq_s = consts.tile([C, 1], F32, tag="bq")
        bp_s = consts.tile([C, 1], F32, tag="bp")
        eye_s = consts.tile([128, 128], F32, tag="eye")
        eye_b = consts.tile([128, 128], BF16, tag="eye_b")
        pos_dt = F32R if pos_via_matmul else F32
        if pos_via_matmul:
            eye_r = consts.tile([128, 128], F32R, tag="eye_r")
        pos_s = [
            consts.tile([128, 512], pos_dt, tag=f"pos{b}", name=f"pos_sb{b}")
            for b in range(B_loc)
        ]
        nc.sync.dma_start(out=w_qs, in_=w_q[:, :])
        nc.sync.dma_start(out=w_ks, in_=w_k[:, :, :])
        nc.sync.dma_start(out=w_vs, in_=w_v[:, :])
        nc.sync.dma_start(out=w_ps_, in_=w_p[:, :])
        nc.sync.dma_start(out=bq_s, in_=bq_d[:, :])
        nc.sync.dma_start(out=bp_s, in_=bp_d[:, :])
        nc.sync.dma_start(out=eye_s, in_=eye_d[:, :])
        nc.gpsimd.dma_start(out=eye_b, in_=eye_d[:, :])
        if pos_via_matmul:
            nc.gpsimd.dma_start(out=eye_r, in_=eye_d[:, :])
        for b in range(B_loc):
            nc.gpsimd.dma_start(out=pos_s[b], in_=pos[b])

        for b, i in rows:
            # ---- load row (16 windows), casting fp32 -> bf16 in-DMA ----
            q_sb = qkv_p.tile([128, nW * L], BF16, tag="q")
            k_sb = qkv_p.tile([128, nW * L], BF16, tag="k")
            v_sb = qkv_p.tile([128, nW * L], BF16, tag="v")
            nc.gpsimd.dma_start(out=q_sb, in_=qT[b, i])
            nc.gpsimd.dma_start(out=k_sb, in_=kT[b, i])
            nc.gpsimd.dma_start(out=v_sb, in_=vT[b, i])

            # ---- projections ----
            qpT = projo.tile([128, nW * L], BF16, tag="qpT")
            kpT_m = [
                projo.tile([128, nW * L], BF16, tag=f"kpT{m}", name=f"kpT_m{m}")
                for m in range(2)
            ]
            vp = projo.tile([128, nW * L], BF16, tag="vp")
            for hf in range(2):
                sl = slice(hf * 512, hf * 512 + 512)
                qp_ps = ps_proj.tile([128, 512], F32, tag="pp")
                nc.tensor.matmul(qp_ps, w_qs, q_sb[:, sl], start=True, stop=True)
                nc.scalar.activation(
                    qpT[:, sl], qp_ps, mybir.ActivationFunctionType.Identity,
                    bias=bq_s,
                )
                for m in range(2):
                    kp_ps = ps_proj.tile([128, 512], F32, tag="pp")
                    nc.tensor.matmul(
                        kp_ps, w_ks[:, m, :], k_sb[:, sl], start=True, stop=True
                    )
                    nc.scalar.activation(
                        kpT_m[m][:, sl], kp_ps, mybir.ActivationFunctionType.Copy
                    )
            for g in range(2):  # vp: natural layout, per pair of windows
                vp_ps = ps_proj.tile([128, 512], F32, tag="pp")
                for p in range(4):
                    pair = g * 4 + p
                    nc.tensor.matmul(
                        vp_ps[:, p * 128 : p * 128 + 128],
                        v_sb[:, pair * 128 : pair * 128 + 128],
                        w_vs,
                        start=True,
                        stop=True,
                    )
                nc.vector.tensor_copy(vp[:, g * 512 : g * 512 + 512], vp_ps)

            attnT_t = {}
            for t in range(4):  # 2-pair S tiles (windows 4t..4t+3)
                s_ps = ps_s.tile([128, 512], F32, tag="s")
                if pos_via_matmul:
                    # position pre-load (f32r identity matmul, full bank)
                    nc.tensor.matmul(
                        s_ps,
                        eye_r,
                        pos_s[b],
                        start=True,
                        stop=False,
                    )
                # S layout: p = 64*hB + lq, f = p2*256 + 128*hL + 64*w + lv,
                # head h = 2*hB + hL. All matmuls on diagonal 64x64 tiles:
                # K=64 (two heads stacked), masked k-weights kill the
                # sibling head's contribution.
                for hB in range(2):
                    rp = slice(64 * hB, 64 * hB + 64)
                    for hL in range(2):
                        for p2 in range(2):
                            pair = 2 * t + p2
                            for w in range(2):
                                is_start = (
                                    not pos_via_matmul
                                    and hL == 0
                                    and p2 == 0
                                    and w == 0
                                )
                                is_stop = hL == 1 and p2 == 1 and w == 1
                                csl = slice(
                                    pair * 128 + w * 64, pair * 128 + w * 64 + 64
                                )
                                out = s_ps[
                                    rp,
                                    p2 * 256 + 128 * hL + 64 * w : p2 * 256
                                    + 128 * hL
                                    + 64 * w
                                    + 64,
                                ]
                                nc.tensor.matmul(
                                    out,
                                    qpT[rp, csl],
                                    kpT_m[hL][rp, csl],
                                    start=is_start,
                                    stop=is_stop,
                                    skip_group_check=True,
                                    tile_position=(64 * hB, 64 * hB),
                                )
                # softmax (no max subtraction: |logits| <~ 25, exp safe in fp32)
                at = attn_p.tile([128, 512], F32, tag="at")
                if pos_via_matmul:
                    nc.scalar.activation(at, s_ps, mybir.ActivationFunctionType.Exp)
                else:
                    nc.vector.tensor_add(at, s_ps, pos_s[b])
                    nc.scalar.activation(at, at, mybir.ActivationFunctionType.Exp)
                sums = small.tile([128, 8], F32, tag="sums")
                nc.vector.reduce_sum(
                    out=sums,
                    in_=at.rearrange("p (g l) -> p g l", l=64),
                    axis=mybir.AxisListType.X,
                )
                recip = small.tile([128, 8], F32, tag="recip")
                nc.vector.reciprocal(recip, sums)
                # one multiply with the reciprocal broadcast along lv
                rb = bass.AP(
                    tensor=recip.tensor,
                    offset=recip.offset,
                    ap=[recip.ap[0], recip.ap[1], [0, 64]],
                )
                at3 = at.rearrange("p (g l) -> p g l", l=64)
                nc.vector.tensor_mul(at3, at3, rb)
                # attn out DMA: tile-contiguous, host unscrambles
                nc.sync.dma_start(out=attn_o[b, i, t], in_=at)
                # transpose attn for AV
                at_ps = ps_at.tile([128, 512], F32, tag="atp")
                for cix in range(4):
                    sl = slice(cix * 128, cix * 128 + 128)
                    nc.tensor.transpose(at_ps[:, sl], at[:, sl], eye_s)
                aT = atT_p.tile([128, 512], BF16, tag="aT")
                nc.scalar.activation(aT, at_ps, mybir.ActivationFunctionType.Copy)
                attnT_t[t] = aT

            for g in range(2):  # AV + out-proj per 4 pairs (8 windows)
                # AV with x in natural layout: diagonal 64x64 tiles
                # (32-wide column tiling is broken on this toolchain/hw)
                x_ps = ps_x.tile([128, 512], F32, tag="x")
                for p4 in range(4):
                    pair = 4 * g + p4
                    t, p2 = pair // 2, pair % 2
                    aT = attnT_t[t]
                    for w in range(2):
                        for h in range(4):
                            lhsT = aT[
                                64 * w : 64 * w + 64,
                                128 * (2 * p2 + h % 2)
                                + 64 * (h // 2) : 128 * (2 * p2 + h % 2)
                                + 64 * (h // 2)
                                + 64,
                            ]
                            rhs = vp[
                                64 * w : 64 * w + 64,
                                pair * 128 + 32 * h : pair * 128 + 32 * h + 32,
                            ]
                            out = x_ps[
                                64 * w : 64 * w + 64,
                                p4 * 128 + 32 * h : p4 * 128 + 32 * h + 32,
                            ]
                            nc.tensor.matmul(
                                out,
                                lhsT,
                                rhs,
                                start=(p4 == 0 and h == 0),
                                stop=(p4 == 3 and h == 3),
                                skip_group_check=True,
                                tile_position=(64 * w, 64 * w),
                            )
                x_sb = xsb_p.tile([128, 512], BF16, tag="x_sb")
                nc.vector.tensor_copy(x_sb, x_ps)
                # transpose x -> feature-major xT for the output projection
                xt_ps = ps_x.tile([128, 512], BF16, tag="x")
                for p4 in range(4):
                    sl = slice(p4 * 128, p4 * 128 + 128)
                    nc.tensor.transpose(xt_ps[:, sl], x_sb[:, sl], eye_b)
                xT_sb = xsb_p.tile([128, 512], BF16, tag="xT")
                nc.scalar.activation(
                    xT_sb, xt_ps, mybir.ActivationFunctionType.Copy
                )
                xo_ps = ps_x.tile([128, 512], F32, tag="x")
                nc.tensor.matmul(xo_ps, w_ps_, xT_sb, start=True, stop=True)
                xo_sb = xout_p.tile([128, 512], F32, tag="xo")
                nc.scalar.activation(
                    xo_sb, xo_ps, mybir.ActivationFunctionType.Identity, bias=bp_s
                )
                nc.sync.dma_start(
                    out=xT_o[b, i, :, g * 512 : g * 512 + 512], in_=xo_sb
                )

    nc.compile()
    return nc


def _get_nc():
    if "nc" not in _compiled:
        _compiled["nc"] = _build()
    return _compiled["nc"]


def _prep_inputs(query, key, value, position, Wq, bq, Wk, Wv, bv, Wp, bp, logit_scale):
    """Host-side prep -> list of per-core input maps."""
    scale = np.exp(np.minimum(np.asarray(logit_scale, np.float32), LOGIT_MAX)).reshape(
        H
    )  # [H]
    col_scale = np.repeat(scale, d)  # per C_out column
    WqT_s = (np.asarray(Wq, np.float32).T * col_scale[None, :]).astype(BF)
    bq_s = (np.asarray(bq, np.float32) * col_scale).astype(np.float32).reshape(C, 1)
    WkT_f = np.asarray(Wk, np.float32).T
    head_par = (np.arange(C) // d) % 2  # parity of each C_out column's head
    WkT = np.stack(
        [WkT_f * (head_par == m)[None, :] for m in range(2)], axis=1
    ).astype(BF)  # [C_in, 2, C_out]
    WvT = np.asarray(Wv, np.float32).T.astype(BF)
    WpT = np.asarray(Wp, np.float32).T.astype(BF)
    bp_eff = (
        (np.asarray(bp, np.float32) + np.asarray(Wp, np.float32) @ np.asarray(bv, np.float32))
        .astype(np.float32)
        .reshape(C, 1)
    )
    eye = np.eye(128, dtype=np.float32)

    def featmaj(x):  # [B,nH,nW,L,C] -> [B,nH,C,nW*L]
        return (
            np.ascontiguousarray(np.asarray(x, np.float32).transpose(0, 1, 4, 2, 3))
            .reshape(B, nH, C, nW * L)
        )

    qT = featmaj(query)
    kT = featmaj(key)
    vT = featmaj(value)

    # position -> [B, (hB lq)=128, (p2 hL w lv)=512], head h = 2*hB + hL
    p4 = np.asarray(position, np.float32).reshape(B, 2, 2, L, L)  # b, hB, hL, lq, lv
    core = p4.transpose(0, 1, 3, 2, 4).reshape(B, 128, 2, 1, L)  # b,(hB lq),hL,w,lv
    core = np.broadcast_to(core, (B, 128, 2, 2, L)).reshape(B, 128, 256)
    pos_arr = np.ascontiguousarray(np.tile(core, (1, 1, 2)))  # [B,128,512]

    in_maps = []
    for c in range(NCORES):
        bs = slice(c * B_loc, (c + 1) * B_loc)
        in_maps.append(
            {
                "qT": np.ascontiguousarray(qT[bs]),
                "kT": np.ascontiguousarray(kT[bs]),
                "vT": np.ascontiguousarray(vT[bs]),
                "pos": np.ascontiguousarray(pos_arr[bs]),
                "w_q": WqT_s,
                "w_k": WkT,
                "w_v": WvT,
                "w_p": WpT,
                "bq_s": bq_s,
                "bp_e": bp_eff,
                "eye": eye,
            }
        )
    return in_maps


def _assemble(results):
    """Per-core outputs -> full (x, attn)."""
    attn_tmp = np.concatenate([r["attn_o"] for r in results], axis=0)
    xT = np.concatenate([r["xT_o"] for r in results], axis=0)
    # attn_tmp [B,nH,t,(hB lq),(p2 hL w lv)] -> [B,nH,(t p2 w),(hB hL),lq,lv]
    a9 = attn_tmp.reshape(B, nH, 4, 2, L, 2, 2, 2, L)  # b,i,t,hB,lq,p2,hL,w,lv
    attn = np.ascontiguousarray(
        a9.transpose(0, 1, 2, 5, 7, 3, 6, 4, 8)  # b,i,t,p2,w,hB,hL,lq,lv
    ).reshape(B, nH, nW, H, L, L)
    x = np.ascontiguousarray(
        xT.reshape(B, nH, C, nW, L).transpose(0, 1, 3, 4, 2)
    )
    return x, attn


def run(in_maps, trace=False, tmpdir=None):
    nc = _get_nc()
    return run_bass_kernel_spmd(
        nc, in_maps, core_ids=list(range(NCORES)), trace=trace, tmpdir=tmpdir
    )


def kernel(query, key, value, position, Wq, bq, Wk, Wv, bv, Wp, bp, logit_scale):
    in_maps = _prep_inputs(
        query, key, value, position, Wq, bq, Wk, Wv, bv, Wp, bp, logit_scale
    )
    res = run(in_maps)
    return _assemble(res.results)
